# revision 17
# baseline (speedup 1.0000x reference)
"""Bass/Trainium2 kernel for nn_BilinearPairedLayer.

Math (per batch b):
  xl = concat([x, shift_down(x,1), shift_up(x,1)], -1)      # [N, 192]
  xr = concat([x, shift_up(x,1), shift_down(x,1)], -1)
  hl = relu(xl @ W_l.T + b_l)                               # [N, 128]
  hr = relu(xr @ W_r.T + b_r)
  out[i,j,k] = sum_g (hl @ W_bil[k])[i,g] * hr[j,g] + b_bil[k]   # [N, N, 2]

Sharding: data-parallel over B — core c computes batch b=c (B=8, 8 cores).
The host-side shard step also re-lays-out the inputs: x arrives transposed
with zero guard columns ([64, 2+1024+2]) and W_l/W_r arrive as per-chunk
lhsT tiles [64, 3, 128], so the device never transposes anything.

Per-core dataflow:
  - all matmuls run in float32r (the TRN2 fp32 fast-matmul format, ~12-bit
    effective mantissa, ~2.5x faster than plain fp32). The fp32r operands
    are fed RAW from fp32 host data via f32r-typed DRAM tensors — the PE
    rounds internally (verified bit-identical error to pre-rounded input),
    so no on-chip cast pass is needed
  - fp32r matmuls do NOT register as PE-busy for the HAM clock gate, so
    small plain-fp32 heartbeat matmuls are woven through the whole kernel
    to keep the PE at 2.4 GHz; a warmup spinner fills the pre-input window
  - the context shifts are free: shifted feature chunks of xl^T are
    column-offset views of xT thanks to the guard columns, so hlT/hrT
    accumulate 3 full 512-column matmuls per N-chunk
  - hlT/hrT [128, N] = relu(W @ xlT + b) with per-partition ACT bias
  - tTk [128, N] = W_bil[k] (stationary, as stored) @ hlT
  - out tile [i=128, j=512] = tTk[:, iblk].T @ hrT chunk  (PSUM fp32)
  - b_bil + (j,k)-interleave fused into the PSUM->SBUF copy: DVE
    tensor_scalar_add writes k=0 stride-2 columns, ACT activation
    (Identity, per-partition bias) writes k=1 — then a 512 KiB DMA per
    half row-block; emission ordered so the first DMA starts early.
"""

import numpy as np

B, N, NIN = 8, 1024, 64
H = 128
NOUT = 2
NCH = 512  # matmul free-dim chunk (one PSUM bank of fp32)
GD = 2     # zero guard columns on each side of xT
NWARM = 4

_cached = {}


def _build():
    import concourse.bacc as bacc
    import concourse.mybir as mybir
    import concourse.tile as tile

    f32 = mybir.dt.float32
    f32r = mybir.dt.float32r
    AF = mybir.ActivationFunctionType

    nc = bacc.Bacc("TRN2", target_bir_lowering=False, debug=False, num_devices=8)

    xt_d = nc.dram_tensor("x_t", [NIN, N + 2 * GD], f32r, kind="ExternalInput").ap()
    wlt_d = nc.dram_tensor("w_lt", [NIN, 3, H], f32r, kind="ExternalInput").ap()
    bl_d = nc.dram_tensor("b_l", [H], f32, kind="ExternalInput").ap()
    wrt_d = nc.dram_tensor("w_rt", [NIN, 3, H], f32r, kind="ExternalInput").ap()
    br_d = nc.dram_tensor("b_r", [H], f32, kind="ExternalInput").ap()
    wb_d = nc.dram_tensor("w_bil", [NOUT, H, H], f32r, kind="ExternalInput").ap()
    bb_d = nc.dram_tensor("b_bil", [NOUT], f32, kind="ExternalInput").ap()
    out_d = nc.dram_tensor("out", [N, N, NOUT], f32, kind="ExternalOutput").ap()
    # row-block view: [8 blocks, 128 rows, (j,k) interleaved 2048]
    out_v = out_d.rearrange("(t p) n k -> t p (n k)", p=128)

    with tile.TileContext(nc) as tc:
        with (
            tc.tile_pool(name="const", bufs=1) as const,
            tc.tile_pool(name="ps", bufs=7, space="PSUM") as ps,
            tc.tile_pool(name="wps", bufs=1, space="PSUM") as wps_pool,
            tc.tile_pool(name="ob", bufs=8) as ob,
        ):
            # ---- input DMAs, most critical first; x split so the first
            # half's completion lands earlier
            XSPLIT = NCH + 2 * GD
            xT = const.tile([NIN, N + 2 * GD], f32r)
            nc.sync.dma_start(out=xT[:, 0:XSPLIT], in_=xt_d[:, 0:XSPLIT])
            nc.sync.dma_start(out=xT[:, XSPLIT:], in_=xt_d[:, XSPLIT:])
            wlT = const.tile([NIN, 3, H], f32r)
            nc.sync.dma_start(out=wlT, in_=wlt_d)
            wrT = const.tile([NIN, 3, H], f32r)
            nc.sync.dma_start(out=wrT, in_=wrt_d)
            wb0 = const.tile([H, H], f32r)
            nc.gpsimd.dma_start(out=wb0, in_=wb_d[0])
            wb1 = const.tile([H, H], f32r)
            nc.gpsimd.dma_start(out=wb1, in_=wb_d[1])
            bl_s = const.tile([H, 1], f32)
            nc.gpsimd.dma_start(out=bl_s, in_=bl_d.unsqueeze(1))
            br_s = const.tile([H, 1], f32)
            nc.gpsimd.dma_start(out=br_s, in_=br_d.unsqueeze(1))
            bb_s = const.tile([128, NOUT], f32)
            nc.gpsimd.dma_start(
                out=bb_s, in_=bb_d.unsqueeze(0).broadcast_to([128, NOUT])
            )

            # ---- PE warmup spinner + HAM heartbeats
            warm = const.tile([128, 256], f32)
            nc.vector.memset(warm, 0.0)
            # dummy ACT ops on a scratch tile: pull the lazy ACT table load
            # to the front without adding deps on `warm`
            actscratch = const.tile([1, 4], f32)
            nc.scalar.activation(actscratch[0:1, 0:2], warm[0:1, 0:2], AF.Relu)
            nc.scalar.activation(actscratch[0:1, 2:4], warm[0:1, 0:2], AF.Identity)
            wps = wps_pool.tile([128, NCH], f32, tag="warm")

            def warmmm():
                nc.tensor.matmul(
                    wps[:, 0:256], warm[:, 0:128], warm,
                    start=True, stop=True, skip_group_check=True,
                )

            def heartbeat():
                # fp32r matmuls don't count as PE-busy for the HAM clock
                # gate; a small plain-fp32 matmul keeps the PE at 2.4 GHz
                nc.tensor.matmul(
                    wps[:, 0:128], warm[:, 0:128], warm[:, 0:128],
                    start=True, stop=True, skip_group_check=True,
                )

            for _ in range(NWARM):
                warmmm()

            hlT = const.tile([H, N], f32r)
            hrT = const.tile([H, N], f32r)

            def h_chunk(dst, wt, bias, s1, j0, hb=False):
                # chunk 1 is shift_down (src col i-1) for xl, shift_up (i+1) for xr
                ph = ps.tile([128, NCH], f32, tag="ps")
                for c, s in ((0, 0), (1, s1), (2, -s1)):
                    nc.tensor.matmul(
                        ph[:, :],
                        wt[:, c, :],
                        xT[:, GD + j0 + s : GD + j0 + s + NCH],
                        start=(c == 0), stop=(c == 2),
                    )
                if hb:
                    heartbeat()
                nc.scalar.activation(
                    dst[:, j0 : j0 + NCH], ph[:, :], AF.Relu,
                    bias=bias[:, 0:1], scale=1.0,
                )

            tT0 = const.tile([H, N], f32r)
            tT1 = const.tile([H, N], f32r)

            def t_chunk(wb, tT, j0, on_act=False, hb=False):
                pt = ps.tile([128, NCH], f32, tag="ps")
                nc.tensor.matmul(
                    pt[:, :], wb, hlT[:, j0 : j0 + NCH], start=True, stop=True
                )
                if hb:
                    heartbeat()
                if on_act:
                    nc.scalar.copy(tT[:, j0 : j0 + NCH], pt[:, :])
                else:
                    nc.vector.tensor_copy(tT[:, j0 : j0 + NCH], pt[:, :])

            def out_half(iblk, j0):
                heartbeat()
                ohalf = ob.tile([128, 2 * NCH], f32, tag="ob")
                for k, tT in ((0, tT0), (1, tT1)):
                    po = ps.tile([128, NCH], f32, tag="ps")
                    nc.tensor.matmul(
                        po[:, :],
                        tT[:, iblk * 128 : (iblk + 1) * 128],
                        hrT[:, j0 : j0 + NCH],
                        start=True, stop=True,
                    )
                    dst = ohalf[:, k : 2 * NCH : 2]
                    if k == 0:
                        nc.vector.tensor_scalar_add(dst, po[:, :], bb_s[:, 0:1])
                    else:
                        nc.scalar.activation(
                            dst, po[:, :], AF.Identity, bias=bb_s[:, 1:2], scale=1.0
                        )
                nc.sync.dma_start(
                    out=out_v[iblk][:, 2 * j0 : 2 * j0 + 2 * NCH], in_=ohalf
                )

            # emission order: earliest first output DMA, PE kept dense.
            # Ramp chunks run inside the warmup's HAM window — no heartbeats.
            h_chunk(hlT, wlT, bl_s, -1, 0)
            h_chunk(hrT, wrT, br_s, +1, 0)
            h_chunk(hlT, wlT, bl_s, -1, NCH)   # PE filler while relu0 lands
            h_chunk(hrT, wrT, br_s, +1, NCH, hb=True)
            t_chunk(wb0, tT0, 0)
            t_chunk(wb1, tT1, 0, on_act=True)  # parallel with tT0 on DVE
            for iblk in range(4):
                out_half(iblk, 0)
            t_chunk(wb0, tT0, NCH, hb=True)
            t_chunk(wb1, tT1, NCH)
            for iblk in range(4):
                out_half(iblk, NCH)
            for iblk in range(4, 8):
                out_half(iblk, 0)
                out_half(iblk, NCH)

    nc.finalize()
    return nc


def make_in_maps(x_l, W_l, b_l, W_r, b_r, W_bil, b_bil):
    # host-side layout: W chunks to lhsT [f=64, chunk, h], x to [64, N] with
    # zero guard columns
    def w_chunks(W):
        return np.ascontiguousarray(
            np.asarray(W, np.float32).reshape(H, 3, NIN).transpose(2, 1, 0)
        )

    x_l = np.asarray(x_l, np.float32)
    xt = np.zeros((B, NIN, N + 2 * GD), np.float32)
    xt[:, :, GD : GD + N] = x_l.transpose(0, 2, 1)

    com = {
        "w_lt": w_chunks(W_l),
        "b_l": np.ascontiguousarray(b_l, np.float32),
        "w_rt": w_chunks(W_r),
        "b_r": np.ascontiguousarray(b_r, np.float32),
        "w_bil": np.ascontiguousarray(W_bil, np.float32),
        "b_bil": np.ascontiguousarray(b_bil, np.float32),
    }
    return [{"x_t": np.ascontiguousarray(xt[c]), **com} for c in range(B)]


def kernel(x_l, W_l, b_l, W_r, b_r, W_bil, b_bil):
    from concourse import bass_utils

    if "nc" not in _cached:
        _cached["nc"] = _build()
    nc = _cached["nc"]

    in_maps = make_in_maps(x_l, W_l, b_l, W_r, b_r, W_bil, b_bil)
    res = bass_utils.run_bass_kernel_spmd(nc, in_maps, core_ids=list(range(B)))
    return np.stack([res.results[c]["out"] for c in range(B)], axis=0)


# revision 18
# speedup vs baseline: 1.1076x; 1.1076x over previous
"""Bass/Trainium2 kernel for nn_BilinearPairedLayer.

Math (per batch b):
  xl = concat([x, shift_down(x,1), shift_up(x,1)], -1)      # [N, 192]
  xr = concat([x, shift_up(x,1), shift_down(x,1)], -1)
  hl = relu(xl @ W_l.T + b_l)                               # [N, 128]
  hr = relu(xr @ W_r.T + b_r)
  out[i,j,k] = sum_g (hl @ W_bil[k])[i,g] * hr[j,g] + b_bil[k]   # [N, N, 2]

Sharding: data-parallel over B — core c computes batch b=c (B=8, 8 cores).
The host-side shard step also re-lays-out the inputs: x arrives transposed
with zero guard columns ([64, 2+1024+2]) and W_l/W_r arrive as per-chunk
lhsT tiles [64, 3, 128], so the device never transposes anything.

Per-core dataflow:
  - all matmuls run in float32r (the TRN2 fp32 fast-matmul format, ~12-bit
    effective mantissa, ~2.5x faster than plain fp32). The fp32r operands
    are fed RAW from fp32 host data via f32r-typed DRAM tensors — the PE
    rounds internally (verified bit-identical error to pre-rounded input),
    so no on-chip cast pass is needed
  - fp32r matmuls do NOT register as PE-busy for the HAM clock gate, so
    small plain-fp32 heartbeat matmuls are woven through the whole kernel
    to keep the PE at 2.4 GHz; a warmup spinner fills the pre-input window
  - the context shifts are free: shifted feature chunks of xl^T are
    column-offset views of xT thanks to the guard columns, so hlT/hrT
    accumulate 3 full 512-column matmuls per N-chunk
  - hlT/hrT [128, N] = relu(W @ xlT + b) with per-partition ACT bias
  - tTk [128, N] = W_bil[k] (stationary, as stored) @ hlT
  - out tile [i=128, j=512] = tTk[:, iblk].T @ hrT chunk  (PSUM fp32)
  - b_bil + (j,k)-interleave fused into the PSUM->SBUF copy: DVE
    tensor_scalar_add writes k=0 stride-2 columns, ACT activation
    (Identity, per-partition bias) writes k=1 — then a 512 KiB DMA per
    half row-block; emission ordered so the first DMA starts early.
"""

import numpy as np

B, N, NIN = 8, 1024, 64
H = 128
NOUT = 2
NCH = 512  # matmul free-dim chunk (one PSUM bank of fp32)
GD = 2     # zero guard columns on each side of xT
NWARM = 7

_cached = {}


def _build():
    import concourse.bacc as bacc
    import concourse.mybir as mybir
    import concourse.tile as tile

    f32 = mybir.dt.float32
    f32r = mybir.dt.float32r
    AF = mybir.ActivationFunctionType

    nc = bacc.Bacc("TRN2", target_bir_lowering=False, debug=False, num_devices=8)

    xt_d = nc.dram_tensor("x_t", [NIN, N + 2 * GD], f32r, kind="ExternalInput").ap()
    wlt_d = nc.dram_tensor("w_lt", [NIN, 3, H], f32r, kind="ExternalInput").ap()
    bl_d = nc.dram_tensor("b_l", [H], f32, kind="ExternalInput").ap()
    wrt_d = nc.dram_tensor("w_rt", [NIN, 3, H], f32r, kind="ExternalInput").ap()
    br_d = nc.dram_tensor("b_r", [H], f32, kind="ExternalInput").ap()
    wb_d = nc.dram_tensor("w_bil", [NOUT, H, H], f32r, kind="ExternalInput").ap()
    bb_d = nc.dram_tensor("b_bil", [NOUT], f32, kind="ExternalInput").ap()
    out_d = nc.dram_tensor("out", [N, N, NOUT], f32, kind="ExternalOutput").ap()
    # row-block view: [8 blocks, 128 rows, (j,k) interleaved 2048]
    out_v = out_d.rearrange("(t p) n k -> t p (n k)", p=128)

    with tile.TileContext(nc) as tc:
        with (
            tc.tile_pool(name="const", bufs=1) as const,
            tc.tile_pool(name="ps", bufs=7, space="PSUM") as ps,
            tc.tile_pool(name="wps", bufs=1, space="PSUM") as wps_pool,
            tc.tile_pool(name="ob", bufs=8) as ob,
        ):
            # ---- input DMAs, most critical first; x split so the first
            # half's completion lands earlier
            XSPLIT = NCH + 2 * GD
            xT = const.tile([NIN, N + 2 * GD], f32r)
            nc.sync.dma_start(out=xT[:, 0:XSPLIT], in_=xt_d[:, 0:XSPLIT])
            nc.sync.dma_start(out=xT[:, XSPLIT:], in_=xt_d[:, XSPLIT:])
            wlT = const.tile([NIN, 3, H], f32r)
            nc.sync.dma_start(out=wlT, in_=wlt_d)
            wrT = const.tile([NIN, 3, H], f32r)
            nc.sync.dma_start(out=wrT, in_=wrt_d)
            wb0 = const.tile([H, H], f32r)
            nc.gpsimd.dma_start(out=wb0, in_=wb_d[0])
            wb1 = const.tile([H, H], f32r)
            nc.gpsimd.dma_start(out=wb1, in_=wb_d[1])
            bl_s = const.tile([H, 1], f32)
            nc.gpsimd.dma_start(out=bl_s, in_=bl_d.unsqueeze(1))
            br_s = const.tile([H, 1], f32)
            nc.gpsimd.dma_start(out=br_s, in_=br_d.unsqueeze(1))
            bb_s = const.tile([128, NOUT], f32)
            nc.gpsimd.dma_start(
                out=bb_s, in_=bb_d.unsqueeze(0).broadcast_to([128, NOUT])
            )

            # ---- PE warmup spinner + HAM heartbeats
            warm = const.tile([128, 256], f32)
            nc.vector.memset(warm, 0.0)
            # dummy ACT ops on a scratch tile: pull the lazy ACT table load
            # to the front without adding deps on `warm`
            actscratch = const.tile([1, 4], f32)
            nc.scalar.activation(actscratch[0:1, 0:2], warm[0:1, 0:2], AF.Relu)
            nc.scalar.activation(actscratch[0:1, 2:4], warm[0:1, 0:2], AF.Identity)
            wps = wps_pool.tile([128, NCH], f32, tag="warm")

            def warmmm():
                nc.tensor.matmul(
                    wps[:, 0:256], warm[:, 0:128], warm,
                    start=True, stop=True, skip_group_check=True,
                )

            def heartbeat():
                # fp32r matmuls don't count as PE-busy for the HAM clock
                # gate; a small plain-fp32 matmul keeps the PE at 2.4 GHz
                nc.tensor.matmul(
                    wps[:, 0:128], warm[:, 0:128], warm[:, 0:128],
                    start=True, stop=True, skip_group_check=True,
                )

            for _ in range(NWARM):
                warmmm()

            hlT = const.tile([H, N], f32r)
            hrT = const.tile([H, N], f32r)

            def h_chunk(dst, wt, bias, s1, j0, hb=False):
                # chunk 1 is shift_down (src col i-1) for xl, shift_up (i+1) for xr
                ph = ps.tile([128, NCH], f32, tag="ps")
                for c, s in ((0, 0), (1, s1), (2, -s1)):
                    nc.tensor.matmul(
                        ph[:, :],
                        wt[:, c, :],
                        xT[:, GD + j0 + s : GD + j0 + s + NCH],
                        start=(c == 0), stop=(c == 2),
                    )
                if hb:
                    heartbeat()
                nc.scalar.activation(
                    dst[:, j0 : j0 + NCH], ph[:, :], AF.Relu,
                    bias=bias[:, 0:1], scale=1.0,
                )

            tT0 = const.tile([H, N], f32r)
            tT1 = const.tile([H, N], f32r)

            def t_chunk(wb, tT, j0, on_act=False, hb=False):
                pt = ps.tile([128, NCH], f32, tag="ps")
                nc.tensor.matmul(
                    pt[:, :], wb, hlT[:, j0 : j0 + NCH], start=True, stop=True
                )
                if hb:
                    heartbeat()
                if on_act:
                    nc.scalar.copy(tT[:, j0 : j0 + NCH], pt[:, :])
                else:
                    nc.vector.tensor_copy(tT[:, j0 : j0 + NCH], pt[:, :])

            def out_half(iblk, j0):
                heartbeat()
                ohalf = ob.tile([128, 2 * NCH], f32, tag="ob")
                for k, tT in ((0, tT0), (1, tT1)):
                    po = ps.tile([128, NCH], f32, tag="ps")
                    nc.tensor.matmul(
                        po[:, :],
                        tT[:, iblk * 128 : (iblk + 1) * 128],
                        hrT[:, j0 : j0 + NCH],
                        start=True, stop=True,
                    )
                    dst = ohalf[:, k : 2 * NCH : 2]
                    if k == 0:
                        nc.vector.tensor_scalar_add(dst, po[:, :], bb_s[:, 0:1])
                    else:
                        nc.scalar.activation(
                            dst, po[:, :], AF.Identity, bias=bb_s[:, 1:2], scale=1.0
                        )
                nc.sync.dma_start(
                    out=out_v[iblk][:, 2 * j0 : 2 * j0 + 2 * NCH], in_=ohalf
                )

            # emission order: earliest first output DMA, PE kept dense.
            # Ramp chunks run inside the warmup's HAM window — no heartbeats.
            h_chunk(hlT, wlT, bl_s, -1, 0)
            h_chunk(hrT, wrT, br_s, +1, 0)
            h_chunk(hlT, wlT, bl_s, -1, NCH)   # PE filler while relu0 lands
            h_chunk(hrT, wrT, br_s, +1, NCH, hb=True)
            t_chunk(wb0, tT0, 0)
            t_chunk(wb1, tT1, 0, on_act=True)  # parallel with tT0 on DVE
            for iblk in range(4):
                out_half(iblk, 0)
            t_chunk(wb0, tT0, NCH, hb=True)
            t_chunk(wb1, tT1, NCH)
            for iblk in range(4):
                out_half(iblk, NCH)
            for iblk in range(4, 8):
                out_half(iblk, 0)
                out_half(iblk, NCH)

    nc.finalize()
    return nc


def make_in_maps(x_l, W_l, b_l, W_r, b_r, W_bil, b_bil):
    # host-side layout: W chunks to lhsT [f=64, chunk, h], x to [64, N] with
    # zero guard columns
    def w_chunks(W):
        return np.ascontiguousarray(
            np.asarray(W, np.float32).reshape(H, 3, NIN).transpose(2, 1, 0)
        )

    x_l = np.asarray(x_l, np.float32)
    xt = np.zeros((B, NIN, N + 2 * GD), np.float32)
    xt[:, :, GD : GD + N] = x_l.transpose(0, 2, 1)

    com = {
        "w_lt": w_chunks(W_l),
        "b_l": np.ascontiguousarray(b_l, np.float32),
        "w_rt": w_chunks(W_r),
        "b_r": np.ascontiguousarray(b_r, np.float32),
        "w_bil": np.ascontiguousarray(W_bil, np.float32),
        "b_bil": np.ascontiguousarray(b_bil, np.float32),
    }
    return [{"x_t": np.ascontiguousarray(xt[c]), **com} for c in range(B)]


def kernel(x_l, W_l, b_l, W_r, b_r, W_bil, b_bil):
    from concourse import bass_utils

    if "nc" not in _cached:
        _cached["nc"] = _build()
    nc = _cached["nc"]

    in_maps = make_in_maps(x_l, W_l, b_l, W_r, b_r, W_bil, b_bil)
    res = bass_utils.run_bass_kernel_spmd(nc, in_maps, core_ids=list(range(B)))
    return np.stack([res.results[c]["out"] for c in range(B)], axis=0)
